# revision 1
# baseline (speedup 1.0000x reference)
"""Trainium2 Bass kernel: masked-LSTM readout over to_dense_batch'd graphs.

Strategy (8 NeuronCores, SPMD single program):
 - Host: per-graph lengths from sorted `index`; graphs globally sorted by
   length (desc) and dealt round-robin to 8 cores, so all cores share one
   step schedule N_t = ceil(#active_global(t)/8). Host densifies x into a
   block-major padded tensor per core (bf16).
 - Device: per time-block, DMA-xbar-transpose loads x-dense as
   [128 = feat + 64*(t%2), cols]; per step, 4 accumulating bf16 matmul
   pairs compute gate preactivations for the active column prefix,
   ScalarE applies sigmoid/tanh (bias folded in), VectorE does the cell
   update, and each graph's final h is snapshotted into an output tile
   via a predicated copy at its true last step.
 - Host: gather per-core outputs, invert the deal/sort permutation.
"""

import numpy as np
import ml_dtypes

MAXLEN = 100
B = 8192
NCORES = 8
G = B // NCORES          # graph columns per core
H = 64
F = 64
TW = 20                  # steps per time block (even)
CHUNK = 512              # matmul free-dim chunk (psum bank)

_CACHE = {}


def _build_and_compile(schedule, weights):
    """Build the Bass program for a given (global) schedule. Returns nc."""
    import concourse.bacc as bacc
    import concourse.mybir as mybir
    from concourse import tile

    N_t, blocks, snap = schedule  # N_t: list; blocks: [(t0, nsteps, Wb, row0)]; snap: [(lo, hi, moff)]
    (wfi_x, wfi_h), (wog_x, wog_h), b_fi, b_og, sc_og = weights
    bf16 = mybir.dt.bfloat16
    f32 = mybir.dt.float32
    T_end = len(N_t)
    ROWS_TOT = sum(Wb * nst // 2 for (_, nst, Wb, _) in blocks)
    MW = sum(hi - lo for pieces in snap for (_, lo, hi, _) in pieces)
    XT_W = max(Wb * nst // 2 for (_, nst, Wb, _) in blocks)

    nc = bacc.Bacc("TRN2", target_bir_lowering=False)
    xd_d = nc.dram_tensor("xd", [128, ROWS_TOT], bf16, kind="ExternalInput")
    msk_d = nc.dram_tensor("msk", [64, max(MW, 1)], mybir.dt.uint8, kind="ExternalInput")
    out_d = nc.dram_tensor("outh", [64, G], bf16, kind="ExternalOutput")

    wfix_d = nc.dram_tensor("wfix", [128, 128], bf16, kind="ExternalInput")
    wogx_d = nc.dram_tensor("wogx", [128, 128], bf16, kind="ExternalInput")
    wfih_d = nc.dram_tensor("wfih", [64, 128], bf16, kind="ExternalInput")
    wogh_d = nc.dram_tensor("wogh", [64, 128], bf16, kind="ExternalInput")
    bfi_d = nc.dram_tensor("bfi", [128, 1], f32, kind="ExternalInput")
    bog_d = nc.dram_tensor("bog", [128, 1], f32, kind="ExternalInput")
    scog_d = nc.dram_tensor("scog", [128, 1], f32, kind="ExternalInput")

    Sig = mybir.ActivationFunctionType.Sigmoid
    Tanh = mybir.ActivationFunctionType.Tanh
    Mult = mybir.AluOpType.mult
    Add = mybir.AluOpType.add

    with tile.TileContext(nc) as tc:
        with tc.tile_pool(name="state", bufs=1) as sp, \
             tc.tile_pool(name="xblk", bufs=2) as xp, \
             tc.tile_pool(name="psum", bufs=2, space="PSUM") as pp:
            wfix = sp.tile([128, 128], bf16)
            nc.sync.dma_start(out=wfix, in_=wfix_d.ap())
            wogx = sp.tile([128, 128], bf16)
            nc.sync.dma_start(out=wogx, in_=wogx_d.ap())
            wfih = sp.tile([64, 128], bf16)
            nc.sync.dma_start(out=wfih, in_=wfih_d.ap())
            wogh = sp.tile([64, 128], bf16)
            nc.sync.dma_start(out=wogh, in_=wogh_d.ap())
            bfi = sp.tile([128, 1], f32)
            nc.sync.dma_start(out=bfi, in_=bfi_d.ap())
            bog = sp.tile([128, 1], f32)
            nc.sync.dma_start(out=bog, in_=bog_d.ap())
            scog = sp.tile([128, 1], f32)
            nc.sync.dma_start(out=scog, in_=scog_d.ap())
            mskt = sp.tile([64, max(MW, 1)], mybir.dt.uint8)
            nc.sync.dma_start(out=mskt, in_=msk_d.ap())

            h, cg, sfi, so, tc_t, fc, ig, outh = ({} for _ in range(8))
            for k in range(2):
                h[k] = sp.tile([64, CHUNK], bf16, tag=f"h{k}", name=f"h{k}")
                cg[k] = sp.tile([64, CHUNK], f32, tag=f"cg{k}", name=f"cg{k}")
                sfi[k] = sp.tile([128, CHUNK], f32, tag=f"sfi{k}", name=f"sfi{k}")
                so[k] = sp.tile([128, CHUNK], f32, tag=f"so{k}", name=f"so{k}")
                tc_t[k] = sp.tile([64, CHUNK], f32, tag=f"tc{k}", name=f"tc{k}")
                fc[k] = sp.tile([64, CHUNK], f32, tag=f"fc{k}", name=f"fc{k}")
                ig[k] = sp.tile([64, CHUNK], f32, tag=f"ig{k}", name=f"ig{k}")
                outh[k] = sp.tile([64, CHUNK], bf16, tag=f"oh{k}", name=f"oh{k}")
                nc.vector.memset(h[k][:, :], 0.0)
                nc.vector.memset(cg[k][:, :], 0.0)
                nc.vector.memset(outh[k][:, :], 0.0)

            for (t0, nsteps, Wb, row0) in blocks:
                rows_b = Wb * nsteps // 2
                xt = xp.tile([128, XT_W], bf16, tag="xt")
                nc.sync.dma_start(
                    out=xt[:, 0:rows_b], in_=xd_d.ap()[:, row0:row0 + rows_b])

                for ts in range(nsteps):
                    t = t0 + ts
                    n = N_t[t]
                    if n == 0:
                        continue
                    par = ts % 2
                    # work items: (psum_tag, state_tile, p0, p1); tail steps
                    # split the lone chunk into two pieces on separate psum
                    # banks so their ACT/DVE chains can interleave
                    if n > CHUNK:
                        work = [(0, 0, 0, CHUNK), (1, 1, 0, n - CHUNK)]
                    elif n >= 128:
                        m = (n // 2 + 1) & ~1
                        work = [(0, 0, 0, m), (1, 0, m, n)]
                    else:
                        work = [(0, 0, 0, n)]
                    fi_ps, og_ps = {}, {}
                    for (kt, km, p0, p1) in work:
                        w = p1 - p0
                        c0 = CHUNK * km + p0
                        fi_ps[kt] = pp.tile([128, CHUNK], f32, tag=f"fi{kt}", name=f"fi{kt}")
                        og_ps[kt] = pp.tile([128, CHUNK], f32, tag=f"og{kt}", name=f"og{kt}")
                        xs = xt[par * 64:(par + 1) * 64,
                                ts // 2 * Wb + c0:
                                ts // 2 * Wb + c0 + w]
                        nc.tensor.matmul(out=fi_ps[kt][:, 0:w],
                                         lhsT=wfix[par * 64:(par + 1) * 64, :],
                                         rhs=xs, start=True, stop=False)
                        nc.tensor.matmul(out=fi_ps[kt][:, 0:w],
                                         lhsT=wfih[:, :],
                                         rhs=h[km][:, p0:p1], start=False, stop=True)
                        nc.tensor.matmul(out=og_ps[kt][:, 0:w],
                                         lhsT=wogx[par * 64:(par + 1) * 64, :],
                                         rhs=xs, start=True, stop=False)
                        nc.tensor.matmul(out=og_ps[kt][:, 0:w],
                                         lhsT=wogh[:, :],
                                         rhs=h[km][:, p0:p1], start=False, stop=True)
                    for (kt, km, p0, p1) in work:
                        w = p1 - p0
                        nc.scalar.activation(out=sfi[km][:, p0:p1], in_=fi_ps[kt][:, 0:w],
                                             func=Sig, bias=bfi[:, :])
                        nc.scalar.activation(out=so[km][:, p0:p1], in_=og_ps[kt][:, 0:w],
                                             func=Sig, bias=bog[:, :], scale=scog[:, :])
                    for (kt, km, p0, p1) in work:
                        nc.vector.scalar_tensor_tensor(
                            out=fc[km][:, p0:p1], in0=cg[km][:, p0:p1], scalar=0.0,
                            in1=sfi[km][0:64, p0:p1], op0=Add, op1=Mult)
                        nc.vector.scalar_tensor_tensor(
                            out=ig[km][:, p0:p1], in0=so[km][64:128, p0:p1], scalar=-0.5,
                            in1=sfi[km][64:128, p0:p1], op0=Add, op1=Mult)
                        nc.vector.scalar_tensor_tensor(
                            out=cg[km][:, p0:p1], in0=ig[km][:, p0:p1], scalar=2.0,
                            in1=fc[km][:, p0:p1], op0=Mult, op1=Add)
                    for (kt, km, p0, p1) in work:
                        nc.scalar.activation(out=tc_t[km][:, p0:p1], in_=cg[km][:, p0:p1], func=Tanh)
                        nc.vector.tensor_tensor(out=h[km][:, p0:p1], in0=so[km][0:64, p0:p1],
                                                in1=tc_t[km][:, p0:p1], op=Mult)
                    for (kk, lo, hi, moff) in snap[t]:
                        nc.vector.copy_predicated(
                            out=outh[kk][:, lo:hi],
                            mask=mskt[:, moff:moff + (hi - lo)],
                            data=h[kk][:, lo:hi])

            nc.sync.dma_start(out=out_d.ap()[:, 0:CHUNK], in_=outh[0][:, :])
            nc.sync.dma_start(out=out_d.ap()[:, CHUNK:G], in_=outh[1][:, :])
    nc.compile()
    return nc


def _plan(lens):
    """Global schedule from capped lengths [B]. Returns (order, schedule helpers)."""
    order = np.argsort(-lens, kind="stable")
    lens_sorted = lens[order]
    T_end = int(lens_sorted.max())
    # per-core sorted lengths: core c, col j -> lens_sorted[8j + c]
    len_c = lens_sorted.reshape(G, NCORES).T  # [NCORES, G]
    # n_c(t) = #cols with len > t
    t_ax = np.arange(T_end + 1)
    n_c = (len_c[:, :, None] > t_ax[None, None, :]).sum(axis=1)  # [NCORES, T_end+1]
    N_t = n_c.max(axis=0)  # [T_end+1]; N_t[T_end] == 0
    # time blocks
    blocks = []
    row0 = 0
    t0 = 0
    while t0 < T_end:
        nsteps = min(TW, T_end - t0)
        if nsteps % 2:
            nsteps += 1  # keep even; schedule N_t beyond T_end is 0-pad
        Wb = int(np.ceil(N_t[t0] / 16) * 16)
        blocks.append((t0, nsteps, Wb, row0))
        row0 += Wb * nsteps // 2
        t0 += nsteps
    # snapshot ranges + masks
    snap = []
    moff = 0
    mask_cols = []
    for t in range(T_end):
        nt1 = n_c[:, t + 1] if t + 1 <= T_end else np.zeros(NCORES, np.int64)
        lo = int(nt1.min())
        hi = int(n_c[:, t].max())
        pieces = []
        if hi > lo:
            m = np.zeros((NCORES, hi - lo), np.uint8)
            for c in range(NCORES):
                a, b_ = int(nt1[c]), int(n_c[c, t])
                m[c, max(a - lo, 0):max(b_ - lo, 0)] = 1
            mask_cols.append(m)
            for k in range(2):
                plo = max(lo, 512 * k)
                phi = min(hi, 512 * (k + 1))
                if phi > plo:
                    pieces.append((k, plo - 512 * k, phi - 512 * k,
                                   moff + (plo - lo)))
            moff += hi - lo
        snap.append(pieces)
    masks = (np.concatenate(mask_cols, axis=1) if mask_cols
             else np.zeros((NCORES, 1), np.uint8))
    # pad schedule for block overhang (nsteps even rounding)
    N_pad = list(N_t[:T_end])
    total_steps = sum(ns for (_, ns, _, _) in blocks)
    while len(N_pad) < total_steps:
        N_pad.append(0)
        snap.append([])
    # drop zero-width steps from the tail of the schedule
    sched_N = [int(x) for x in N_pad]
    return order, len_c, n_c, sched_N, blocks, snap, masks


LAST_RUN = {}


def _install_ntff_shim():
    import sys, types
    if "antenv.axon_hooks" in sys.modules:
        return
    try:
        from trn_agent_boot.trn_boot import _ntff_profile_via_ctypes
        hook = _ntff_profile_via_ctypes("/opt/axon/libaxon_pjrt.so")
    except Exception:
        hook = None
    m = types.ModuleType("antenv.axon_hooks")
    m._hook = hook
    m.get_axon_ntff_profile_hook = lambda: m._hook
    m.set_axon_ntff_profile_hook = lambda h: setattr(m, "_hook", h)
    sys.modules["antenv.axon_hooks"] = m


def kernel(x, W_ih, W_hh, b_ih, b_hh, index, dim_size, _trace=False):
    from concourse.bass_utils import run_bass_kernel_spmd
    if _trace:
        import concourse.bass_utils as _bu
        _install_ntff_shim()
        _bu.upload_artifacts = lambda d: d  # no bucket in this container

    x = np.asarray(x)
    index = np.asarray(index).astype(np.int64)
    W_ih = np.asarray(W_ih, dtype=np.float32)
    W_hh = np.asarray(W_hh, dtype=np.float32)
    b_ih = np.asarray(b_ih, dtype=np.float32)
    b_hh = np.asarray(b_hh, dtype=np.float32)

    assert int(dim_size) == B, f"kernel hardcodes B={B}, got dim_size={int(dim_size)}"
    counts = np.bincount(index, minlength=B).astype(np.int64)
    offsets = np.concatenate([[0], np.cumsum(counts)[:-1]])
    lens = np.minimum(counts, MAXLEN)

    order, len_c, n_c, N_t, blocks, snap, masks = _plan(lens)

    # --- weights (torch gate order i,f,g,o -> ours f,i / o,g) ---
    b = (b_ih + b_hh).reshape(4, H)
    Wi, Wf, Wg, Wo = W_ih.reshape(4, H, F)
    Ui, Uf, Ug, Uo = W_hh.reshape(4, H, H)
    bf16 = ml_dtypes.bfloat16

    # ih stationaries duplicated at both parity halves (x-slices alternate
    # partition halves); hh stationaries at parts 0:64 (h lives there).
    wfi_x = np.concatenate([np.concatenate([Wf.T, Wi.T], 1)] * 2, 0).astype(bf16)
    wog_x = np.concatenate([np.concatenate([Wo.T, Wg.T], 1)] * 2, 0).astype(bf16)
    wfi_h = np.concatenate([Uf.T, Ui.T], 1).astype(bf16)  # [64, 128]
    wog_h = np.concatenate([Uo.T, Ug.T], 1).astype(bf16)
    b_fi = np.concatenate([b[1], b[0]]).reshape(128, 1).astype(np.float32)
    b_og = np.concatenate([b[3], 2.0 * b[2]]).reshape(128, 1).astype(np.float32)
    sc_og = np.concatenate([np.ones(64), 2.0 * np.ones(64)]).reshape(128, 1).astype(np.float32)

    # --- per-core dense input (block-major) ---
    x_bf = x.astype(bf16)
    T_end = len(N_t)
    in_maps = []
    for c in range(NCORES):
        gids = order[np.arange(G) * NCORES + c]     # col j -> graph id
        lens_cj = len_c[c]                          # [G]
        offs_cj = offsets[gids]
        parts = []
        for (t0, nsteps, Wb, row0) in blocks:
            tsl = np.arange(t0, t0 + nsteps)
            node = offs_cj[:Wb, None] + tsl[None, :]             # [Wb, nsteps]
            valid = tsl[None, :] < lens_cj[:Wb, None]
            node = np.clip(node, 0, x.shape[0] - 1)
            blk = np.where(valid[:, :, None], x_bf[node], bf16(0))  # [Wb, nsteps, 64]
            # time-major rows: row r = taupair*Wb + g  -> per-step rhs contiguous
            blk = blk.reshape(Wb, nsteps // 2, 128).transpose(1, 0, 2)
            parts.append(blk.reshape(nsteps // 2 * Wb, 128))
        xd = np.ascontiguousarray(np.concatenate(parts, axis=0).T)
        msk = np.ascontiguousarray(
            np.broadcast_to(masks[c][None, :], (64, masks.shape[1])))
        in_maps.append({"xd": xd, "msk": msk,
                        "wfix": wfi_x, "wogx": wog_x, "wfih": wfi_h,
                        "wogh": wog_h, "bfi": b_fi, "bog": b_og, "scog": sc_og})

    key = (tuple(N_t), tuple(blocks), repr(snap),
           W_ih.tobytes(), W_hh.tobytes(), b_ih.tobytes(), b_hh.tobytes())
    import hashlib
    key = hashlib.sha1(repr(key[:3]).encode() + key[3] + key[4] + key[5] + key[6]).hexdigest()
    if key not in _CACHE:
        _CACHE[key] = _build_and_compile(
            (N_t, blocks, snap),
            ((wfi_x, wfi_h), (wog_x, wog_h), b_fi, b_og, sc_og))
    nc = _CACHE[key]

    res = run_bass_kernel_spmd(nc, in_maps, core_ids=list(range(NCORES)),
                               trace=_trace)
    LAST_RUN["res"] = res

    out = np.zeros((B, H), np.float32)
    for c in range(NCORES):
        hT = res.results[c]["outh"].astype(np.float32)  # [64, G]
        gids = order[np.arange(G) * NCORES + c]
        out[gids] = hT.T
    return out



# revision 10
# speedup vs baseline: 1.0626x; 1.0626x over previous
"""Trainium2 Bass kernel: masked-LSTM readout over to_dense_batch'd graphs.

v2 strategy (8 NeuronCores, SPMD single program):
 - Host: per-graph lengths from sorted `index`; graphs globally sorted by
   length (desc) and dealt round-robin to 8 cores, so all cores share one
   step schedule N_t. Host densifies x into a block-major padded tensor per
   core (fp16), feature-major [64, rows].
 - Device per step: rhs = [x_t ; H_{t-1}] stacked on 128 partitions (H==2h
   written into the x tile's bottom half by the previous step), so each
   gate-pair needs ONE matmul with contract 128. Two independent column
   pieces pipeline the serial chain. Gates: sigmoid ACT for (f,i); tanh ACT
   with per-partition scale (0.5;1) for (o,g) giving o~=2sig(o)-1 and
   g~=tanh(g) directly. Cell update in fp16 on DVE: 3 tensor_tensor (2x
   mode) + 1 scalar_tensor_tensor; tanh(c) on ACT. Final h snapshot via
   predicated copy at each graph's last valid step.
 - Host: gather per-core outputs (H=2h -> h), invert the deal permutation.
"""

import numpy as np

MAXLEN = 100
B = 8192
NCORES = 8
G = B // NCORES          # graph columns per core
H = 64
F = 64
TW = 20                  # steps per time block
CHUNK = 512              # psum bank width (f32 cols)

_CACHE = {}


def _build_and_compile(schedule, weights):
    import concourse.bacc as bacc
    import concourse.mybir as mybir
    from concourse import tile

    N_t, blocks, snap, MW = schedule
    wfi_np, wog_np, bfi_np, bog_np, scog_np = weights
    fp16 = mybir.dt.float16
    f32 = mybir.dt.float32
    T_end = len(N_t)
    ROWS_TOT = sum(Wb * nst for (_, nst, Wb, _) in blocks)
    XT_W = max(Wb * nst for (_, nst, Wb, _) in blocks)

    nc = bacc.Bacc("TRN2", target_bir_lowering=False)
    xd_d = nc.dram_tensor("xd", [64, ROWS_TOT], fp16, kind="ExternalInput")
    msk_d = nc.dram_tensor("msk", [64, max(MW, 1)], mybir.dt.uint8, kind="ExternalInput")
    out_d = nc.dram_tensor("outh", [64, G], fp16, kind="ExternalOutput")
    wfi_d = nc.dram_tensor("wfi", [128, 128], fp16, kind="ExternalInput")
    wog_d = nc.dram_tensor("wog", [128, 128], fp16, kind="ExternalInput")
    bfi_d = nc.dram_tensor("bfi", [128, 1], f32, kind="ExternalInput")
    bog_d = nc.dram_tensor("bog", [128, 1], f32, kind="ExternalInput")
    scog_d = nc.dram_tensor("scog", [128, 1], f32, kind="ExternalInput")

    Sig = mybir.ActivationFunctionType.Sigmoid
    Tanh = mybir.ActivationFunctionType.Tanh
    Mult = mybir.AluOpType.mult
    Add = mybir.AluOpType.add

    # block index for each step, and column base within the block's tile
    blk_of = {}
    for bi, (t0, nst, Wb, row0) in enumerate(blocks):
        for ts in range(nst):
            blk_of[t0 + ts] = (bi, ts)

    with tile.TileContext(nc) as tc:
        with tc.tile_pool(name="state", bufs=1) as sp, \
             tc.tile_pool(name="xblk", bufs=2) as xp, \
             tc.tile_pool(name="psum", bufs=2, space="PSUM") as pp, \
             tc.tile_pool(name="gates", bufs=2) as gp:
            wfi = sp.tile([128, 128], fp16)
            nc.sync.dma_start(out=wfi, in_=wfi_d.ap())
            wog = sp.tile([128, 128], fp16)
            nc.sync.dma_start(out=wog, in_=wog_d.ap())
            bfi = sp.tile([128, 1], f32)
            nc.sync.dma_start(out=bfi, in_=bfi_d.ap())
            bog = sp.tile([128, 1], f32)
            nc.sync.dma_start(out=bog, in_=bog_d.ap())
            scog = sp.tile([128, 1], f32)
            nc.sync.dma_start(out=scog, in_=scog_d.ap())
            mskt = sp.tile([64, max(MW, 1)], mybir.dt.uint8)
            nc.sync.dma_start(out=mskt, in_=msk_d.ap())

            c = sp.tile([64, 1024], fp16, name="c")
            t1 = sp.tile([64, 1024], fp16, name="t1")
            t2 = sp.tile([64, 1024], fp16, name="t2")
            tcc = sp.tile([64, 1024], fp16, name="tcc")
            hs = sp.tile([64, 1024], fp16, name="hs")
            outh = sp.tile([64, 1024], fp16, name="outh")
            nc.vector.memset(c[:, :], 0.0)
            nc.vector.memset(outh[:, :], 0.0)
            nc.vector.memset(hs[:, :], 0.0)

            xts = {}  # live xt tiles by block index

            def make_xt(bi2):
                if bi2 in xts or bi2 >= len(blocks):
                    return
                _, nst2, Wb2, row02 = blocks[bi2]
                rows2 = Wb2 * nst2
                xt2 = xp.tile([128, XT_W], fp16, tag="xt", name=f"xt{bi2}")
                xts[bi2] = xt2
                # x occupies partitions 64:128; H (=2h) occupies 0:64
                nc.sync.dma_start(out=xt2[64:128, 0:rows2],
                                  in_=xd_d.ap()[:, row02:row02 + rows2])

            for bi, (t0, nsteps, Wb, row0) in enumerate(blocks):
                make_xt(bi)
                make_xt(bi + 1)  # prefetch next block's x
                xt = xts[bi]
                if bi == 0:
                    nc.vector.memset(xt[0:64, 0:Wb], 0.0)

                for ts in range(nsteps):
                    t = t0 + ts
                    n = N_t[t]
                    if n == 0:
                        continue
                    base = ts * Wb
                    # independent column pieces (each <= CHUNK wide)
                    if n > CHUNK:
                        pieces = [(0, CHUNK), (CHUNK, n)]
                    elif n > 32:
                        m = (n // 2 + 15) & ~15
                        pieces = [(0, m), (m, n)]
                    else:
                        pieces = [(0, n)]

                    # next-step destination for H
                    if t + 1 < T_end:
                        nbi, nts = blk_of[t + 1]
                        Wb_n = blocks[nbi][2]
                        base_n = nts * Wb_n
                        wA = min(n, Wb_n)
                        xt_n = xts[nbi]
                    else:
                        wA = 0
                        xt_n = None

                    sgf, sgo, psA, psB = {}, {}, {}, {}
                    for pi, (lo, hi) in enumerate(pieces):
                        w = hi - lo
                        ps = pp.tile([128, 1024], mybir.dt.float32,
                                     tag=f"ps{pi}", name=f"ps{pi}")
                        (psA if pi == 0 else psB)[0] = ps
                        nc.tensor.matmul(out=ps[:, 0:w], lhsT=wfi[:, :],
                                         rhs=xt[:, base + lo:base + hi],
                                         start=True, stop=True)
                        nc.tensor.matmul(out=ps[:, CHUNK:CHUNK + w], lhsT=wog[:, :],
                                         rhs=xt[:, base + lo:base + hi],
                                         start=True, stop=True)
                    for pi, (lo, hi) in enumerate(pieces):
                        w = hi - lo
                        ps = (psA if pi == 0 else psB)[0]
                        sg = gp.tile([128, 1024], fp16, tag=f"sg{pi}", name=f"sg{pi}")
                        sgf[pi] = sg
                        nc.scalar.activation(out=sg[:, 0:w], in_=ps[:, 0:w],
                                             func=Sig, bias=bfi[:, :])
                        nc.scalar.activation(out=sg[:, CHUNK:CHUNK + w],
                                             in_=ps[:, CHUNK:CHUNK + w],
                                             func=Tanh, bias=bog[:, :],
                                             scale=scog[:, :])
                    for pi, (lo, hi) in enumerate(pieces):
                        w = hi - lo
                        sg = sgf[pi]
                        # t1 = sig(f) * c ; t2 = sig(i) * tanh(g) ; c = t1 + t2
                        nc.vector.tensor_tensor(
                            out=t1[:, lo:hi], in0=sg[0:64, 0:w],
                            in1=c[:, lo:hi], op=Mult)
                        nc.vector.tensor_tensor(
                            out=t2[:, lo:hi], in0=sg[64:128, 0:w],
                            in1=sg[64:128, CHUNK:CHUNK + w], op=Mult)
                        nc.vector.tensor_tensor(
                            out=c[:, lo:hi], in0=t1[:, lo:hi],
                            in1=t2[:, lo:hi], op=Add)
                        nc.scalar.activation(out=tcc[:, lo:hi], in_=c[:, lo:hi],
                                             func=Tanh)
                        # H = (o~ + 1) * tanh(c)  == 2h
                        for (a, b_) in _split((lo, hi), wA):
                            wseg = b_ - a
                            if wseg <= 0:
                                continue
                            if b_ <= wA:
                                dst = xt_n[0:64, base_n + a:base_n + b_]
                            else:
                                dst = hs[:, a:b_]
                            nc.vector.scalar_tensor_tensor(
                                out=dst, in0=sg[0:64, CHUNK + a - lo:CHUNK + b_ - lo],
                                scalar=1.0, in1=tcc[:, a:b_], op0=Add, op1=Mult)
                    # snapshot graphs ending at step t
                    for (lo, hi, moff) in snap[t]:
                        for (a, b_) in _split((lo, hi), wA):
                            wseg = b_ - a
                            if wseg <= 0:
                                continue
                            if b_ <= wA:
                                src = xt_n[0:64, base_n + a:base_n + b_]
                            else:
                                src = hs[:, a:b_]
                            nc.vector.copy_predicated(
                                out=outh[:, a:b_],
                                mask=mskt[:, moff + a - lo:moff + b_ - lo],
                                data=src)
            nc.sync.dma_start(out=out_d.ap()[:, 0:G], in_=outh[:, 0:G])
    nc.compile()
    return nc


def _split(rng, cut):
    """Split [lo,hi) at cut into segments lying fully below or above cut."""
    lo, hi = rng
    if cut <= lo:
        return [(lo, hi)]
    if cut >= hi:
        return [(lo, hi)]
    return [(lo, cut), (cut, hi)]


def _plan(lens):
    """Global schedule from capped lengths [B]."""
    order = np.argsort(-lens, kind="stable")
    lens_sorted = lens[order]
    T_end = int(lens_sorted.max())
    len_c = lens_sorted.reshape(G, NCORES).T            # [NCORES, G]
    t_ax = np.arange(T_end + 1)
    n_c = (len_c[:, :, None] > t_ax[None, None, :]).sum(axis=1)
    N_t = n_c.max(axis=0)                               # [T_end+1], N_t[T_end]==0
    blocks = []
    row0 = 0
    t0 = 0
    while t0 < T_end:
        nsteps = min(TW, T_end - t0)
        Wb = max(16, int(np.ceil(N_t[t0] / 16) * 16))
        blocks.append((t0, nsteps, Wb, row0))
        row0 += Wb * nsteps
        t0 += nsteps
    snap = []
    moff = 0
    mask_cols = []
    for t in range(T_end):
        nt1 = n_c[:, t + 1]
        lo = int(nt1.min())
        hi = int(n_c[:, t].max())
        pieces = []
        if hi > lo:
            m = np.zeros((NCORES, hi - lo), np.uint8)
            for cc in range(NCORES):
                a, b_ = int(nt1[cc]), int(n_c[cc, t])
                m[cc, max(a - lo, 0):max(b_ - lo, 0)] = 1
            mask_cols.append(m)
            pieces.append((lo, hi, moff))
            moff += hi - lo
        snap.append(pieces)
    masks = (np.concatenate(mask_cols, axis=1) if mask_cols
             else np.zeros((NCORES, 1), np.uint8))
    return order, len_c, n_c, [int(x) for x in N_t[:T_end]], blocks, snap, masks


LAST_RUN = {}


def _install_ntff_shim():
    import sys, types
    if "antenv.axon_hooks" in sys.modules:
        return
    try:
        from trn_agent_boot.trn_boot import _ntff_profile_via_ctypes
        hook = _ntff_profile_via_ctypes("/opt/axon/libaxon_pjrt.so")
    except Exception:
        hook = None
    m = types.ModuleType("antenv.axon_hooks")
    m._hook = hook
    m.get_axon_ntff_profile_hook = lambda: m._hook
    m.set_axon_ntff_profile_hook = lambda h: setattr(m, "_hook", h)
    sys.modules["antenv.axon_hooks"] = m


def kernel(x, W_ih, W_hh, b_ih, b_hh, index, dim_size, _trace=False):
    from concourse.bass_utils import run_bass_kernel_spmd
    if _trace:
        import concourse.bass_utils as _bu
        _install_ntff_shim()
        _bu.upload_artifacts = lambda d: d

    x = np.asarray(x)
    index = np.asarray(index).astype(np.int64)
    W_ih = np.asarray(W_ih, dtype=np.float32)
    W_hh = np.asarray(W_hh, dtype=np.float32)
    b_ih = np.asarray(b_ih, dtype=np.float32)
    b_hh = np.asarray(b_hh, dtype=np.float32)

    assert int(dim_size) == B, f"kernel hardcodes B={B}, got dim_size={int(dim_size)}"
    counts = np.bincount(index, minlength=B).astype(np.int64)
    offsets = np.concatenate([[0], np.cumsum(counts)[:-1]])
    lens = np.minimum(counts, MAXLEN)

    order, len_c, n_c, N_t, blocks, snap, masks = _plan(lens)

    # --- weights (torch gate order i,f,g,o) ---
    b = (b_ih + b_hh).reshape(4, H)
    Wi, Wf, Wg, Wo = W_ih.reshape(4, H, F)
    Ui, Uf, Ug, Uo = W_hh.reshape(4, H, H)
    # rhs rows 0:64 carry H = 2h (h-weights pre-halved); rows 64:128 carry x.
    wfi = np.concatenate(
        [np.concatenate([0.5 * Uf.T, 0.5 * Ui.T], 1),
         np.concatenate([Wf.T, Wi.T], 1)], 0).astype(np.float16)
    wog = np.concatenate(
        [np.concatenate([0.5 * Uo.T, 0.5 * Ug.T], 1),
         np.concatenate([Wo.T, Wg.T], 1)], 0).astype(np.float16)
    # og ACT: tanh(pre*scale + bias); o-rows scale .5 (o~=2sig(o)-1), g scale 1
    bfi = np.concatenate([b[1], b[0]]).reshape(128, 1).astype(np.float32)
    bog = np.concatenate([0.5 * b[3], b[2]]).reshape(128, 1).astype(np.float32)
    scog = np.concatenate([0.5 * np.ones(64), np.ones(64)]).reshape(128, 1).astype(np.float32)

    # --- per-core dense input, feature-major [64, rows] ---
    x16 = x.astype(np.float16)
    in_maps = []
    for cN in range(NCORES):
        gids = order[np.arange(G) * NCORES + cN]
        lens_cj = len_c[cN]
        offs_cj = offsets[gids]
        parts = []
        for (t0, nsteps, Wb, row0) in blocks:
            tsl = np.arange(t0, t0 + nsteps)
            node = offs_cj[:Wb, None] + tsl[None, :]             # [Wb, nsteps]
            valid = tsl[None, :] < lens_cj[:Wb, None]
            node = np.clip(node, 0, x.shape[0] - 1)
            blk = np.where(valid[:, :, None], x16[node], np.float16(0))
            # row r = ts*Wb + col
            parts.append(blk.transpose(1, 0, 2).reshape(nsteps * Wb, 64))
        xd = np.ascontiguousarray(np.concatenate(parts, axis=0).T)
        msk = np.ascontiguousarray(
            np.broadcast_to(masks[cN][None, :], (64, masks.shape[1])))
        in_maps.append({"xd": xd, "msk": msk, "wfi": wfi, "wog": wog,
                        "bfi": bfi, "bog": bog, "scog": scog})

    import hashlib
    key = hashlib.sha1(
        (repr((N_t, blocks, snap)) ).encode()
        + W_ih.tobytes() + W_hh.tobytes() + b_ih.tobytes() + b_hh.tobytes()
    ).hexdigest()
    if key not in _CACHE:
        _CACHE[key] = _build_and_compile(
            (N_t, blocks, snap, masks.shape[1]),
            (wfi, wog, bfi, bog, scog))
    nc = _CACHE[key]

    res = run_bass_kernel_spmd(nc, in_maps, core_ids=list(range(NCORES)),
                               trace=_trace)
    LAST_RUN["res"] = res

    out = np.zeros((B, H), np.float32)
    for cN in range(NCORES):
        hT = res.results[cN]["outh"].astype(np.float32)  # [64, G] == 2h
        gids = order[np.arange(G) * NCORES + cN]
        out[gids] = 0.5 * hT.T
    return out
